# revision 31
# baseline (speedup 1.0000x reference)
"""Trainium2 Bass kernel: GQA attention layer (RoPE + causal attention + projections).

Strategy (8 NeuronCores, tensor-parallel by head):
  - Each core owns 2 query heads + 1 kv head (NH=16, NKV=8 -> GQA pairs align
    with cores exactly). QKV projection, RoPE, and attention for those heads run
    fully locally -- zero K/V communication.
  - Q/K projections run in fp8(e4m3) DoubleRow mode (2 k-tiles packed per PE
    cell, ~1.5x bf16 matmul throughput). Safe here because attention scores are
    tiny (~4e-4): fp8's ~4% relative error on Q/K becomes negligible absolute
    score error. Power-of-2 operand scaling (x64 activations, x256/x64 weights)
    is undone for free inside the softmax exp via the ACT engine's scale
    parameter. V and all value-path matmuls stay bf16 for accuracy.
  - Attention is computed in the S^T orientation ([keys, q]) so the probability
    matrix feeds the PV matmul directly as the moving operand. Softmax
    denominator accumulates via an all-ones stationary matmul; normalization is
    applied after PV. The PV matmul lags its exp by one key-chunk so the ACT
    latency stays off the PE critical path.
  - Zipper schedule: attention strips of group g-1 interleave into group g's
    projection matmul stream, so exp latency hides behind projection matmuls
    instead of head-of-line blocking the in-order PE queue. The last group's
    strips zip with o_proj of batch 0 (pure PE work whose inputs arrive via the
    first AllToAll, long since complete).
  - One AllToAll per batch reshards head-sharded activations to token-sharded;
    o_proj streams its output in 512-column chunks so the final DMA drain is
    short. Weight/table loads ride the second hardware DGE queue (ACT engine)
    so they never stall the activation stream on the SP queue.
"""

import os
from contextlib import ExitStack

import ml_dtypes
import numpy as np

import concourse.bass as bass
import concourse.tile as tile
from concourse import bacc, mybir
from concourse.bass_utils import run_bass_kernel_spmd

# Problem shapes (hardcoded per spec nn_AvaAttention_36249523978775).
B, T, HID = 2, 2048, 2048
NH, NKV, HD = 16, 8, 128
SCALE = HD ** -0.5
NC = 8
TT = B * T  # 4096 flat tokens, b-major
NEG = -2.3819763e38

F32 = mybir.dt.float32
BF = mybir.dt.bfloat16
F8 = mybir.dt.float8e4
NPBF = ml_dtypes.bfloat16
NPF8 = ml_dtypes.float8_e4m3

TN = 512           # token chunk for projection moving operand
NG = TT // TN      # 8 projection token groups
NHC = HID // 128   # 16 contraction chunks (bf16 path)
NHCP = NHC // 2    # 8 contraction pair-chunks (fp8 DoubleRow path)
NQC = T // 256     # 8 query strips of 256 per batch
NKC = T // 128     # 16 key chunks of 128 per batch

# fp8 operand scaling (powers of 2; descaled inside exp)
SC_H = 64.0
SC_QW = 256.0
SC_KW = 64.0
EXP_SCALE = float(1.0 / (SC_H * SC_H * SC_QW * SC_KW))  # 2**-26

DR = mybir.MatmulPerfMode.DoubleRow

_CACHE = {}
last_results = None  # test harness reads exec_time_ns from here


def _build(mode: str):
    """Build the SPMD graph. mode in {"causal", "none", "generic"}."""
    nc = bacc.Bacc("TRN2", target_bir_lowering=False, debug=False, num_devices=NC)

    hT8_e = nc.declare_dram_parameter("hT8", [NG, 128, NHCP, 2, TN], F8, isOutput=False)
    hTb_e = nc.declare_dram_parameter("hTb", [NG, 128, NHC, TN], BF, isOutput=False)
    w8_e = nc.declare_dram_parameter("w8", [128, 3, NHCP, 2, 128], F8, isOutput=False)
    wv_e = nc.declare_dram_parameter("wv", [128, NHC, 128], BF, isOutput=False)
    woT_e = nc.declare_dram_parameter("woT", [NH * HD, HID], BF, isOutput=False)
    ropeC_e = nc.declare_dram_parameter("ropeC", [128, T], BF, isOutput=False)
    ropeS_e = nc.declare_dram_parameter("ropeS", [128, T], BF, isOutput=False)
    ones_e = nc.declare_dram_parameter("ones", [128, 128], BF, isOutput=False)
    ident_e = nc.declare_dram_parameter("ident", [128, 128], BF, isOutput=False)
    pat_e = None
    maskT_e = None
    if mode == "causal":
        pat_e = nc.declare_dram_parameter("pat", [2, 128, 2, 256], F32, isOutput=False)
    elif mode == "generic":
        maskT_e = nc.declare_dram_parameter("maskT", [T, T], F32, isOutput=False)
    out_e = nc.declare_dram_parameter("out", [512, HID], F32, isOutput=True)

    with tile.TileContext(nc) as tc:
        with tc.tile_pool(name="consts", bufs=1) as consts, \
             tc.tile_pool(name="dram", bufs=1, space="DRAM") as dram:

            ones_t = consts.tile([128, 128], BF)
            ident_t = consts.tile([128, 128], BF)
            pat_t = None
            if mode == "causal":
                pat_t = consts.tile([128, 2, 2, 256], F32)

            # per-token-half tensors (collective inputs must be contiguous)
            # so the AllToAlls can ship token halves as separate collectives
            a2a_in = [[dram.tile([NC, 256, 128], BF, name=f"a2a_in{b}h{u}")
                       for u in range(2)] for b in range(B)]
            a2a_out = [[dram.tile([NC, 256, 128], BF, name=f"a2a_out{b}h{u}")
                        for u in range(2)] for b in range(B)]

            # o_proj weights + attention-gather tiles: reserved early
            # (pool-nesting order), DMA emitted later.
            es_wo = ExitStack()
            wop = es_wo.enter_context(tc.tile_pool(name="wop", bufs=1))
            wo_res = [wop.tile([128, NH, 1024], BF, name=f"wo{half}")
                      for half in range(2)]
            attgp = es_wo.enter_context(tc.tile_pool(name="attg", bufs=2))

            es = ExitStack()
            big = es.enter_context(tc.tile_pool(name="big", bufs=1))
            # Persistent activations (my heads, all tokens).
            q_sb = big.tile([128, 2, TT], BF)      # Q^T, 2 q heads
            k_sb = big.tile([128, TT], BF)         # K^T, 1 kv head
            v_sb = big.tile([128, TT // 128, 128], BF)  # V natural, [tok-chunk, d]

            att_g = [None, None]
            psF_pool = [None]
            fo_pool_r = [None]

            # -------- Phase A+B zippered: projection feeds attention ---------
            # Attention pools outer (live through the tail); projection-only
            # pools in an inner ExitStack closed after the group loop so psF
            # fits in PSUM (stack allocation is LIFO).
            with tc.tile_pool(name="psS", bufs=3, space="PSUM") as psS, \
                 tc.tile_pool(name="psPV", bufs=1, space="PSUM") as psPV, \
                 tc.tile_pool(name="psDen", bufs=1, space="PSUM") as psDen, \
                 tc.tile_pool(name="pt", bufs=4) as pt_pool, \
                 tc.tile_pool(name="attev", bufs=2) as attev, \
                 tc.tile_pool(name="mt", bufs=3) as mt_pool:
                es_proj = ExitStack()
                wrope = es_proj.enter_context(tc.tile_pool(name="wrope", bufs=1))
                ht8_pool = es_proj.enter_context(tc.tile_pool(name="ht8", bufs=2))
                htb_pool = es_proj.enter_context(tc.tile_pool(name="htb", bufs=2))
                psA = es_proj.enter_context(tc.tile_pool(name="psA", bufs=2, space="PSUM"))
                psTr = es_proj.enter_context(tc.tile_pool(name="psTr", bufs=1, space="PSUM"))
                rtmp = es_proj.enter_context(tc.tile_pool(name="ropetmp", bufs=2))
                vtmp = es_proj.enter_context(tc.tile_pool(name="vtmp", bufs=2))
                ropeC_t = wrope.tile([128, T], BF)
                ropeS_t = wrope.tile([128, T], BF)
                w8_t = wrope.tile([128, 3, NHCP, 2, 128], F8)
                wv_t = wrope.tile([128, NHC, 128], BF)
                # SP queue: fp8 weights first (first matmul needs them), split
                # by slice so s0 matmuls start after 256KB; the group-0 fp8
                # activation stream follows right behind.
                for s in range(3):
                    nc.sync.dma_start(w8_t[:, s], w8_e[:, s])
                # ACT queue (second hwdge stream, idle at startup): rope
                # tables, V weights, constants -- parallel with the SP stream.
                # First-group slices first so group 0's RoPE/V never wait on
                # the full-table transfers.
                nc.scalar.dma_start(ropeC_t[:, 0:TN], ropeC_e[:, 0:TN])
                nc.scalar.dma_start(ropeS_t[:, 0:TN], ropeS_e[:, 0:TN])
                nc.scalar.dma_start(wv_t[:], wv_e[:])
                nc.scalar.dma_start(ident_t[:], ident_e[:])
                nc.scalar.dma_start(ones_t[:], ones_e[:])
                nc.scalar.dma_start(ropeC_t[:, TN:T], ropeC_e[:, TN:T])
                nc.scalar.dma_start(ropeS_t[:, TN:T], ropeS_e[:, TN:T])
                if mode == "causal":
                    nc.scalar.dma_start(
                        pat_t[:], pat_e[:].rearrange("s p h t -> p s h t"))

                def strip_units(b, qc):
                    """Yield emission units (callables) for one attention strip."""
                    cmax = 2 * qc + 2 if mode == "causal" else NKC
                    mv = q_sb[:, :, b * T + 256 * qc: b * T + 256 * qc + 256]
                    pv = psPV.tile([128, 512], F32, name="pv", tag="pv")
                    den = psDen.tile([128, 512], F32, name="den", tag="den")
                    state = {"pt_prev": None, "pt_pair": None}

                    def ci_unit(ci):
                        st = psS.tile([128, 512], F32, name="st", tag="st")
                        nc.tensor.matmul(
                            st[:], k_sb[:, b * T + 128 * ci: b * T + 128 * ci + 128],
                            mv, start=True, stop=True)
                        if mode == "causal" and ci >= 2 * qc:
                            sub = ci - 2 * qc
                            nc.vector.tensor_add(
                                st[:], st[:],
                                pat_t[:, sub, :, :].rearrange("p h t -> p (h t)"))
                        elif mode == "generic":
                            mt = mt_pool.tile([128, 256], F32, name="mt", tag="mt")
                            nc.sync.dma_start(
                                mt[:], maskT_e[128 * ci:128 * ci + 128,
                                               256 * qc:256 * qc + 256])
                            nc.vector.tensor_scalar_mul(st[:], st[:], EXP_SCALE)
                            nc.vector.tensor_add(st[:, 0:256], st[:, 0:256], mt[:])
                            nc.vector.tensor_add(st[:, 256:512], st[:, 256:512], mt[:])
                        pt = pt_pool.tile([128, 512], BF, name="pt", tag="pt")
                        if mode == "generic":
                            nc.scalar.activation(pt[:], st[:],
                                                 mybir.ActivationFunctionType.Exp)
                        else:
                            nc.scalar.activation(pt[:], st[:],
                                                 mybir.ActivationFunctionType.Exp,
                                                 scale=EXP_SCALE)
                        # PV lags one ci so exp latency stays off the PE path.
                        if ci > 0:
                            nc.tensor.matmul(pv[:], v_sb[:, NKC * b + ci - 1, :],
                                             state["pt_prev"][:],
                                             start=(ci == 1), stop=False)
                        # denominator: pair-sum pt chunks on DVE (bf16 2x),
                        # halving the ones-matvec count on the PE
                        if ci % 2 == 0:
                            if ci == cmax - 1:
                                nc.tensor.matmul(den[:], ones_t[:], pt[:],
                                                 start=(ci == 0), stop=True)
                            else:
                                state["pt_pair"] = pt
                        else:
                            pts = pt_pool.tile([128, 512], BF, name="pts", tag="pts")
                            nc.vector.tensor_add(pts[:], state["pt_pair"][:], pt[:])
                            nc.tensor.matmul(den[:], ones_t[:], pts[:],
                                             start=(ci == 1), stop=(ci == cmax - 1))
                        state["pt_prev"] = pt

                    def tail_unit():
                        nc.tensor.matmul(pv[:], v_sb[:, NKC * b + cmax - 1, :],
                                         state["pt_prev"][:],
                                         start=(cmax == 1), stop=True)
                        # den rows are all identical (ones stationary)
                        den_rb = attev.tile([128, 512], F32, name="den_rb", tag="den_rb")
                        nc.vector.reciprocal_approx_fast(den_rb[:], den[:])
                        ao = attev.tile([128, 2, 2, 128], BF, name="ao", tag="ao")
                        nc.vector.tensor_mul(
                            ao[:].rearrange("p h u t -> p (h u t)"), pv[:], den_rb[:])
                        for u2 in range(2):
                            nc.sync.dma_start(
                                a2a_in[b][u2][qc].rearrange("(h p) t -> p h t", p=128),
                                ao[:, :, u2, :])

                    for ci in range(cmax):
                        yield lambda ci=ci: ci_unit(ci)
                    yield tail_unit

                ht8_tiles = [None] * NG

                def issue_ht8(g):
                    h8 = ht8_pool.tile([128, NHCP, 2, TN], F8, name="h8", tag="h8")
                    if g == 0:
                        # finer pieces so group 0's first matmuls start early
                        for q in range(NHCP):
                            nc.sync.dma_start(h8[:, q:q + 1],
                                              hT8_e[g, :, q:q + 1])
                    else:
                        nc.sync.dma_start(h8[:], hT8_e[g])
                    ht8_tiles[g] = h8

                def proj_units(g):
                    """Yield emission units for group g's projections."""
                    t0 = g * TN
                    hbt = [None]

                    def dma_unit():
                        # one batched DMA per tensor per group: the SP queue's
                        # ~585ns per-issue cost serializes fine-grained loads
                        # and starves the ao writes that gate the collectives.
                        # ht8 issues one group AHEAD (fp8 stream is the
                        # critical projection input).
                        if g == 0:
                            issue_ht8(0)
                        if g + 1 < NG:
                            issue_ht8(g + 1)
                        # V-path loads ride the ACT hwdge queue: a second
                        # hardware DMA stream in parallel with the fp8 loads
                        hb = htb_pool.tile([128, NHC, TN], BF, name="hb", tag="hb")
                        if g == 0:
                            for q in range(4):
                                nc.scalar.dma_start(hb[:, 4 * q:4 * q + 4],
                                                    hTb_e[g, :, 4 * q:4 * q + 4])
                        else:
                            nc.scalar.dma_start(hb[:], hTb_e[g])
                        hbt[0] = hb
                        if g in (2, 4):
                            half = (g - 2) // 2
                            nc.sync.dma_start(
                                wo_res[half][:],
                                woT_e[:, half * 1024:(half + 1) * 1024]
                                .rearrange("(h p) n -> p h n", p=128))

                    ctab = g % (T // TN) * TN  # rope table column offset

                    def s_unit(s):
                        ps = psA.tile([128, TN], F32, name="psA", tag="psA")
                        if s < 3:
                            for c in range(NHCP):
                                nc.tensor.matmul(ps[:], w8_t[:, s, c],
                                                 ht8_tiles[g][:, c],
                                                 start=(c == 0), stop=(c == NHCP - 1),
                                                 perf_mode=DR)
                            # RoPE: out = ps*C + rot(ps)*S  (S carries the sign)
                            if s < 2:
                                dst = q_sb[:, s, t0:t0 + TN]
                            else:
                                dst = k_sb[:, t0:t0 + TN]
                            csl = ropeC_t[:, ctab:ctab + TN]
                            ssl = ropeS_t[:, ctab:ctab + TN]
                            # stage psum->bf16 on ACT: frees the psA slot after
                            # one read and lets every DVE op run in 2x 16-bit
                            # mode (~410ns vs ~690ns per [128,512])
                            rs = rtmp.tile([128, TN], BF, name="rs", tag="rs")
                            nc.scalar.copy(rs[:], ps[:])
                            t1 = rtmp.tile([128, TN], BF, name="t1", tag="t1")
                            t2 = rtmp.tile([128, TN], BF, name="t2", tag="t2")
                            # ropeS is stored partition-swapped ([sin, -sin])
                            # so each half-mul's SBUF inputs share a base
                            # partition (walrus NCC_IBIR297 constraint)
                            nc.vector.tensor_mul(t1[:], rs[:], csl)
                            nc.vector.tensor_mul(t2[0:64, :], rs[64:128, :], ssl[64:128, :])
                            nc.vector.tensor_mul(t2[64:128, :], rs[0:64, :], ssl[0:64, :])
                            nc.vector.tensor_add(dst, t1[:], t2[:])
                        else:
                            for hc in range(NHC):
                                nc.tensor.matmul(ps[:], wv_t[:, hc], hbt[0][:, hc],
                                                 start=(hc == 0), stop=(hc == NHC - 1))
                            # V^T -> transpose to V natural via PE (staging
                            # copy on ACT: DVE is the lagging engine and its
                            # backlog gates k_sb/strip tails)
                            vt = vtmp.tile([128, TN], BF, name="vt", tag="vt")
                            nc.scalar.copy(vt[:], ps[:])
                            for j in range(TN // 128):
                                trp = psTr.tile([128, 128], BF, name="trp", tag="trp")
                                nc.tensor.transpose(trp[:], vt[:, j * 128:(j + 1) * 128], ident_t[:])
                                nc.vector.tensor_copy(v_sb[:, g * (TN // 128) + j, :], trp[:])

                    yield dma_unit
                    for s in range(4):
                        yield lambda s=s: s_unit(s)

                def oproj_units(p):
                    """Yield emission units for o_proj of batch p (16 chunks)."""
                    def chunk_unit(tch, ch):
                        ps = psF_pool[0].tile([128, 512], F32, name="fin", tag="fin")
                        half, n2 = ch // 2, ch % 2
                        for h in range(NH):
                            nc.tensor.matmul(
                                ps[:],
                                att_g[p][:, h, tch * 128:(tch + 1) * 128],
                                wo_res[half][:, h, n2 * 512:(n2 + 1) * 512],
                                start=(h == 0), stop=(h == NH - 1))
                        fo = fo_pool_r[0].tile([128, 512], F32, name="fo", tag="fo")
                        if ch % 2 == 0:
                            nc.vector.tensor_copy(fo[:], ps[:])
                        else:
                            nc.scalar.copy(fo[:], ps[:])
                        # alternate output writes across both hwdge queues so
                        # the final drain halves
                        eng = nc.sync if ch % 2 == 0 else nc.scalar
                        eng.dma_start(
                            out_e[p * 256 + tch * 128: p * 256 + (tch + 1) * 128,
                                  (half * 1024 + n2 * 512):(half * 1024 + (n2 + 1) * 512)],
                            fo[:])

                    for tch in range(2):
                        for ch in range(4):
                            yield lambda tch=tch, ch=ch: chunk_unit(tch, ch)

                def zip_emit(primary, secondary):
                    """Interleave two unit streams ~evenly; primary-led."""
                    pu = list(primary)
                    su = list(secondary)
                    emitted = 0
                    for i, u in enumerate(pu):
                        u()
                        want = (i + 1) * len(su) // len(pu)
                        while emitted < want:
                            su[emitted]()
                            emitted += 1
                    while emitted < len(su):
                        su[emitted]()
                        emitted += 1

                def strips_for(g):
                    """Strips unlocked by group g's projections."""
                    if g < 0:
                        return []
                    if mode == "causal":
                        b = g // 4
                        return [(b, 2 * (g % 4)), (b, 2 * (g % 4) + 1)]
                    # non-causal strips read every key chunk of the batch
                    if g == 3:
                        return [(0, qc) for qc in range(NQC)]
                    if g == 7:
                        return [(1, qc) for qc in range(NQC)]
                    return []

                def strip_stream(g):
                    for b, qc in strips_for(g):
                        yield from strip_units(b, qc)

                # Rolling zipper: each group's strips enqueue after its k/v
                # are projected and drain interleaved into the NEXT group's
                # projection stream, keeping the PE queue free of exp stalls
                # while completing strips (and thus the AllToAlls) early.
                pending = []
                for g in range(NG):
                    rate = (len(pending) + 4) // 5
                    for u in proj_units(g):
                        u()
                        for _ in range(min(rate, len(pending))):
                            pending.pop(0)()
                    if g == 4:
                        # drain leftover batch-0 strips, then reshard batch 0
                        while pending:
                            pending.pop(0)()
                        for u2 in range(2):
                            nc.gpsimd.collective_compute(
                                "AllToAll", mybir.AluOpType.bypass,
                                replica_groups=[list(range(NC))],
                                ins=[a2a_in[0][u2][:].opt()],
                                outs=[a2a_out[0][u2][:].opt()])
                    pending.extend(strip_stream(g))
                    if g == NG - 1:
                        # gather batch-0 attention early (SP queue; a2a#0 is
                        # complete well before the queue reaches these)
                        att_g[0] = attgp.tile([128, NH, 256], BF, name="attg0",
                                              tag="attg")
                        for j in range(NC):
                            for u2 in range(2):
                                nc.sync.dma_start(
                                    att_g[0][:, 2 * j:2 * j + 2,
                                             128 * u2:128 * u2 + 128],
                                    a2a_out[0][u2][j].rearrange(
                                        "(h p) t -> p h t", p=128))

                es_proj.close()  # free projection pools (PSUM room for psF)

                # Tail: finish the last strips FIRST (their ao writes gate the
                # second AllToAll, shipped as two token-half collectives so
                # o_proj p1's first half starts while the second flies), with
                # o_proj p0 filling the PE meanwhile.
                with tc.tile_pool(name="psF", bufs=3, space="PSUM") as psF, \
                     tc.tile_pool(name="fo", bufs=3) as fo_pool:
                    psF_pool[0] = psF
                    fo_pool_r[0] = fo_pool
                    for u in pending:
                        u()
                    for u2 in range(2):
                        nc.gpsimd.collective_compute(
                            "AllToAll", mybir.AluOpType.bypass,
                            replica_groups=[list(range(NC))],
                            ins=[a2a_in[1][u2][:].opt()],
                            outs=[a2a_out[1][u2][:].opt()])
                    for u in oproj_units(0):
                        u()
                    att_g[1] = attgp.tile([128, NH, 256], BF, name="attg1", tag="attg")
                    for u2 in range(2):
                        for j in range(NC):
                            nc.sync.dma_start(
                                att_g[1][:, 2 * j:2 * j + 2,
                                         128 * u2:128 * u2 + 128],
                                a2a_out[1][u2][j].rearrange("(h p) t -> p h t", p=128))
                    for u in oproj_units(1):
                        u()

            es.close()
            es_wo.close()

    nc.compile()
    return nc


def _host_prep(hidden_states, freqs_cos, freqs_sin, mask, w_qkv, w_o, kv_write_indices):
    idx = np.asarray(kv_write_indices).astype(np.int64)
    if not np.array_equal(idx, np.arange(T, dtype=np.int64)):
        raise NotImplementedError("kernel specialized for kv_write_indices == arange(T)")

    hs = np.asarray(hidden_states, dtype=np.float32).reshape(TT, HID)
    hsT = hs.T  # [HID, TT]
    # bf16 copy (V path): [HID, TT] -> [NG, 128, NHC, TN] (one DMA per group,
    # 16KB contiguous per partition row)
    hTb = np.ascontiguousarray(
        hsT.reshape(NHC, 128, NG, TN).transpose(2, 1, 0, 3)).astype(NPBF)
    # fp8 copy (Q/K path, x64): [HID, TT] -> [NG, 128, NHCP, 2, TN]
    # element (g, p, c, kt, t) = hsT[128*(2c+kt)+p, g*TN+t] * 64
    h8full = (hsT * SC_H).reshape(NHCP, 2, 128, NG, TN).transpose(3, 2, 0, 1, 4)
    hT8 = np.ascontiguousarray(h8full).astype(NPF8)

    m2 = np.asarray(mask, dtype=np.float32).reshape(T, T)
    tril = np.tril(np.ones((T, T), dtype=bool))
    if not m2.any():
        mode = "none"
    elif (m2[tril] == 0).all() and (m2[~tril] <= -1e30).all():
        mode = "causal"
    else:
        mode = "generic"

    wq = np.asarray(w_qkv, dtype=np.float32)
    woT = np.ascontiguousarray(np.asarray(w_o, dtype=np.float32).T).astype(NPBF)

    def tile_w8(wrows):
        # [128 out, HID] -> [128 hid-in-pair? ...] stationary DoubleRow tiles:
        # element (p, c, kt, m) = wrows[m, 128*(2c+kt)+p]
        return wrows.T.reshape(NHCP, 2, 128, 128).transpose(2, 0, 1, 3)

    w8s = []
    wvs = []
    for c in range(NC):
        q1 = wq[(2 * c) * HD:(2 * c + 1) * HD] * (SCALE * SC_QW)
        q2 = wq[(2 * c + 1) * HD:(2 * c + 2) * HD] * (SCALE * SC_QW)
        k = wq[NH * HD + c * HD: NH * HD + (c + 1) * HD] * SC_KW
        v = wq[(NH + NKV) * HD + c * HD: (NH + NKV) * HD + (c + 1) * HD]
        # [3, NHCP, 2, 128] blocks -> [128 hid, 3, NHCP, 2, 128 out]
        w8s.append(np.ascontiguousarray(
            np.stack([tile_w8(q1), tile_w8(q2), tile_w8(k)], axis=1)
        ).astype(NPF8))
        # V: [128 out, HID] -> [NHC, 128 hid, 128 out] -> [128, NHC, 128]
        wvs.append(np.ascontiguousarray(
            v.T.reshape(NHC, 128, 128).transpose(1, 0, 2)).astype(NPBF))

    cosT = np.asarray(freqs_cos, dtype=np.float32).T  # [64, T]
    sinT = np.asarray(freqs_sin, dtype=np.float32).T
    ropeC = np.ascontiguousarray(np.concatenate([cosT, cosT], axis=0)).astype(NPBF)
    # partition-swapped: rows [0:64] hold +sin (used against x1 -> upper out),
    # rows [64:128] hold -sin (used against x2 -> lower out)
    ropeS = np.ascontiguousarray(np.concatenate([sinT, -sinT], axis=0)).astype(NPBF)

    consts = {
        "ropeC": ropeC,
        "ropeS": ropeS,
        "ones": np.ones((128, 128), NPBF),
        "ident": np.eye(128, dtype=np.float32).astype(NPBF),
    }
    if mode == "causal":
        kr = np.arange(256)[:, None]
        qr = np.arange(256)[None, :]
        pat = np.where(kr <= qr, np.float32(0.0), np.float32(NEG)).astype(np.float32)
        pat = pat.reshape(2, 128, 1, 256).repeat(2, axis=2)  # dup over heads
        consts["pat"] = np.ascontiguousarray(pat)
    elif mode == "generic":
        consts["maskT"] = np.ascontiguousarray(m2.T)

    in_maps = []
    for c in range(NC):
        m = {"hT8": hT8, "hTb": hTb, "w8": w8s[c], "wv": wvs[c], "woT": woT}
        m.update(consts)
        in_maps.append(m)
    return mode, in_maps


def kernel(hidden_states, freqs_cos, freqs_sin, k_cache, v_cache, mask, w_qkv,
           w_o, kv_write_indices):
    # k_cache/v_cache are fully overwritten (kv_write_indices == arange covers
    # every slot), so their incoming contents are irrelevant.
    global last_results
    mode, in_maps = _host_prep(hidden_states, freqs_cos, freqs_sin, mask,
                               w_qkv, w_o, kv_write_indices)
    if mode not in _CACHE:
        _CACHE[mode] = _build(mode)
    nc = _CACHE[mode]

    trace = bool(os.environ.get("BASS_KERNEL_TRACE"))
    res = run_bass_kernel_spmd(nc, in_maps, core_ids=list(range(NC)), trace=trace)
    last_results = res

    final = np.empty((B, T, HID), dtype=np.float32)
    for c in range(NC):
        o = res.results[c]["out"]
        final[0, 256 * c:256 * (c + 1)] = o[0:256]
        final[1, 256 * c:256 * (c + 1)] = o[256:512]
    return final


# revision 32
# speedup vs baseline: 1.0874x; 1.0874x over previous
"""Trainium2 Bass kernel: GQA attention layer (RoPE + causal attention + projections).

Strategy (8 NeuronCores, tensor-parallel by head):
  - Each core owns 2 query heads + 1 kv head (NH=16, NKV=8 -> GQA pairs align
    with cores exactly). QKV projection, RoPE, and attention for those heads run
    fully locally -- zero K/V communication.
  - Q/K projections run in fp8(e4m3) DoubleRow mode (2 k-tiles packed per PE
    cell, ~1.5x bf16 matmul throughput). Safe here because attention scores are
    tiny (~4e-4): fp8's ~4% relative error on Q/K becomes negligible absolute
    score error. Power-of-2 operand scaling (x64 activations, x256/x64 weights)
    is undone for free inside the softmax exp via the ACT engine's scale
    parameter. V and all value-path matmuls stay bf16 for accuracy.
  - Attention is computed in the S^T orientation ([keys, q]) so the probability
    matrix feeds the PV matmul directly as the moving operand. Softmax
    denominator accumulates via an all-ones stationary matmul; normalization is
    applied after PV. The PV matmul lags its exp by one key-chunk so the ACT
    latency stays off the PE critical path.
  - Zipper schedule: attention strips of group g-1 interleave into group g's
    projection matmul stream, so exp latency hides behind projection matmuls
    instead of head-of-line blocking the in-order PE queue. The last group's
    strips zip with o_proj of batch 0 (pure PE work whose inputs arrive via the
    first AllToAll, long since complete).
  - One AllToAll per batch reshards head-sharded activations to token-sharded;
    o_proj streams its output in 512-column chunks so the final DMA drain is
    short. Weight/table loads ride the second hardware DGE queue (ACT engine)
    so they never stall the activation stream on the SP queue.
"""

import os
from contextlib import ExitStack

import ml_dtypes
import numpy as np

import concourse.bass as bass
import concourse.tile as tile
from concourse import bacc, mybir
from concourse.bass_utils import run_bass_kernel_spmd

# Problem shapes (hardcoded per spec nn_AvaAttention_36249523978775).
B, T, HID = 2, 2048, 2048
NH, NKV, HD = 16, 8, 128
SCALE = HD ** -0.5
NC = 8
TT = B * T  # 4096 flat tokens, b-major
NEG = -2.3819763e38

F32 = mybir.dt.float32
BF = mybir.dt.bfloat16
F8 = mybir.dt.float8e4
NPBF = ml_dtypes.bfloat16
NPF8 = ml_dtypes.float8_e4m3

TN = 512           # token chunk for projection moving operand
NG = TT // TN      # 8 projection token groups
NHC = HID // 128   # 16 contraction chunks (bf16 path)
NHCP = NHC // 2    # 8 contraction pair-chunks (fp8 DoubleRow path)
NQC = T // 256     # 8 query strips of 256 per batch
NKC = T // 128     # 16 key chunks of 128 per batch

# fp8 operand scaling (powers of 2; descaled inside exp)
SC_H = 64.0
SC_QW = 256.0
SC_KW = 64.0
EXP_SCALE = float(1.0 / (SC_H * SC_H * SC_QW * SC_KW))  # 2**-26

DR = mybir.MatmulPerfMode.DoubleRow

_CACHE = {}
last_results = None  # test harness reads exec_time_ns from here


def _build(mode: str):
    """Build the SPMD graph. mode in {"causal", "none", "generic"}."""
    nc = bacc.Bacc("TRN2", target_bir_lowering=False, debug=False, num_devices=NC)

    hT8_e = nc.declare_dram_parameter("hT8", [NG, 128, NHCP, 2, TN], F8, isOutput=False)
    hTb_e = nc.declare_dram_parameter("hTb", [NG, 128, NHC, TN], BF, isOutput=False)
    w8_e = nc.declare_dram_parameter("w8", [128, 3, NHCP, 2, 128], F8, isOutput=False)
    wv_e = nc.declare_dram_parameter("wv", [128, NHC, 128], BF, isOutput=False)
    woT_e = nc.declare_dram_parameter("woT", [NH * HD, HID], BF, isOutput=False)
    ropeC_e = nc.declare_dram_parameter("ropeC", [128, T], BF, isOutput=False)
    ropeS_e = nc.declare_dram_parameter("ropeS", [128, T], BF, isOutput=False)
    ones_e = nc.declare_dram_parameter("ones", [128, 128], BF, isOutput=False)
    ident_e = nc.declare_dram_parameter("ident", [128, 128], BF, isOutput=False)
    pat_e = None
    maskT_e = None
    if mode == "causal":
        pat_e = nc.declare_dram_parameter("pat", [2, 128, 2, 256], F32, isOutput=False)
    elif mode == "generic":
        maskT_e = nc.declare_dram_parameter("maskT", [T, T], F32, isOutput=False)
    out_e = nc.declare_dram_parameter("out", [512, HID], F32, isOutput=True)

    with tile.TileContext(nc) as tc:
        with tc.tile_pool(name="consts", bufs=1) as consts, \
             tc.tile_pool(name="dram", bufs=1, space="DRAM") as dram:

            ones_t = consts.tile([128, 128], BF)
            ident_t = consts.tile([128, 128], BF)
            pat_t = None
            if mode == "causal":
                pat_t = consts.tile([128, 2, 2, 256], F32)

            # per-token-half tensors (collective inputs must be contiguous)
            # so the AllToAlls can ship token halves as separate collectives
            a2a_in = [[dram.tile([NC, 256, 128], BF, name=f"a2a_in{b}h{u}")
                       for u in range(2)] for b in range(B)]
            a2a_out = [[dram.tile([NC, 256, 128], BF, name=f"a2a_out{b}h{u}")
                        for u in range(2)] for b in range(B)]

            # o_proj weights + attention-gather tiles: reserved early
            # (pool-nesting order), DMA emitted later.
            es_wo = ExitStack()
            wop = es_wo.enter_context(tc.tile_pool(name="wop", bufs=1))
            wo_res = [wop.tile([128, NH, 1024], BF, name=f"wo{half}")
                      for half in range(2)]
            attgp = es_wo.enter_context(tc.tile_pool(name="attg", bufs=2))

            es = ExitStack()
            big = es.enter_context(tc.tile_pool(name="big", bufs=1))
            # Persistent activations (my heads, all tokens).
            q_sb = big.tile([128, 2, TT], BF)      # Q^T, 2 q heads
            k_sb = big.tile([128, TT], BF)         # K^T, 1 kv head
            v_sb = big.tile([128, TT // 128, 128], BF)  # V natural, [tok-chunk, d]

            att_g = [None, None]
            psF_pool = [None]
            fo_pool_r = [None]

            # -------- Phase A+B zippered: projection feeds attention ---------
            # Attention pools outer (live through the tail); projection-only
            # pools in an inner ExitStack closed after the group loop so psF
            # fits in PSUM (stack allocation is LIFO).
            with tc.tile_pool(name="psS", bufs=3, space="PSUM") as psS, \
                 tc.tile_pool(name="psPV", bufs=1, space="PSUM") as psPV, \
                 tc.tile_pool(name="psDen", bufs=1, space="PSUM") as psDen, \
                 tc.tile_pool(name="pt", bufs=4) as pt_pool, \
                 tc.tile_pool(name="attev", bufs=2) as attev, \
                 tc.tile_pool(name="mt", bufs=3) as mt_pool:
                es_proj = ExitStack()
                wrope = es_proj.enter_context(tc.tile_pool(name="wrope", bufs=1))
                ht8_pool = es_proj.enter_context(tc.tile_pool(name="ht8", bufs=2))
                htb_pool = es_proj.enter_context(tc.tile_pool(name="htb", bufs=2))
                psA = es_proj.enter_context(tc.tile_pool(name="psA", bufs=2, space="PSUM"))
                psTr = es_proj.enter_context(tc.tile_pool(name="psTr", bufs=1, space="PSUM"))
                rtmp = es_proj.enter_context(tc.tile_pool(name="ropetmp", bufs=2))
                vtmp = es_proj.enter_context(tc.tile_pool(name="vtmp", bufs=2))
                ropeC_t = wrope.tile([128, T], BF)
                ropeS_t = wrope.tile([128, T], BF)
                w8_t = wrope.tile([128, 3, NHCP, 2, 128], F8)
                wv_t = wrope.tile([128, NHC, 128], BF)
                # SP queue: fp8 weights first (first matmul needs them), split
                # by slice so s0 matmuls start after 256KB; the group-0 fp8
                # activation stream follows right behind.
                for s in range(3):
                    nc.sync.dma_start(w8_t[:, s], w8_e[:, s])
                # ACT queue (second hwdge stream, idle at startup): rope
                # tables, V weights, constants -- parallel with the SP stream.
                # First-group slices first so group 0's RoPE/V never wait on
                # the full-table transfers.
                nc.scalar.dma_start(ropeC_t[:, 0:TN], ropeC_e[:, 0:TN])
                nc.scalar.dma_start(ropeS_t[:, 0:TN], ropeS_e[:, 0:TN])
                nc.scalar.dma_start(wv_t[:], wv_e[:])
                nc.scalar.dma_start(ident_t[:], ident_e[:])
                nc.scalar.dma_start(ones_t[:], ones_e[:])
                nc.scalar.dma_start(ropeC_t[:, TN:T], ropeC_e[:, TN:T])
                nc.scalar.dma_start(ropeS_t[:, TN:T], ropeS_e[:, TN:T])
                if mode == "causal":
                    nc.scalar.dma_start(
                        pat_t[:], pat_e[:].rearrange("s p h t -> p s h t"))

                def strip_units(b, qc):
                    """Yield emission units (callables) for one attention strip."""
                    cmax = 2 * qc + 2 if mode == "causal" else NKC
                    mv = q_sb[:, :, b * T + 256 * qc: b * T + 256 * qc + 256]
                    pv = psPV.tile([128, 512], F32, name="pv", tag="pv")
                    den = psDen.tile([128, 512], F32, name="den", tag="den")
                    state = {"pt_prev": None, "pt_pair": None}

                    def ci_unit(ci):
                        st = psS.tile([128, 512], F32, name="st", tag="st")
                        nc.tensor.matmul(
                            st[:], k_sb[:, b * T + 128 * ci: b * T + 128 * ci + 128],
                            mv, start=True, stop=True)
                        if mode == "causal" and ci >= 2 * qc:
                            sub = ci - 2 * qc
                            nc.vector.tensor_add(
                                st[:], st[:],
                                pat_t[:, sub, :, :].rearrange("p h t -> p (h t)"))
                        elif mode == "generic":
                            mt = mt_pool.tile([128, 256], F32, name="mt", tag="mt")
                            nc.sync.dma_start(
                                mt[:], maskT_e[128 * ci:128 * ci + 128,
                                               256 * qc:256 * qc + 256])
                            nc.vector.tensor_scalar_mul(st[:], st[:], EXP_SCALE)
                            nc.vector.tensor_add(st[:, 0:256], st[:, 0:256], mt[:])
                            nc.vector.tensor_add(st[:, 256:512], st[:, 256:512], mt[:])
                        pt = pt_pool.tile([128, 512], BF, name="pt", tag="pt")
                        if mode == "generic":
                            nc.scalar.activation(pt[:], st[:],
                                                 mybir.ActivationFunctionType.Exp)
                        else:
                            nc.scalar.activation(pt[:], st[:],
                                                 mybir.ActivationFunctionType.Exp,
                                                 scale=EXP_SCALE)
                        # PV lags one ci so exp latency stays off the PE path.
                        if ci > 0:
                            nc.tensor.matmul(pv[:], v_sb[:, NKC * b + ci - 1, :],
                                             state["pt_prev"][:],
                                             start=(ci == 1), stop=False)
                        # denominator: pair-sum pt chunks on DVE (bf16 2x),
                        # halving the ones-matvec count on the PE
                        if ci % 2 == 0:
                            if ci == cmax - 1:
                                nc.tensor.matmul(den[:], ones_t[:], pt[:],
                                                 start=(ci == 0), stop=True)
                            else:
                                state["pt_pair"] = pt
                        else:
                            pts = pt_pool.tile([128, 512], BF, name="pts", tag="pts")
                            nc.vector.tensor_add(pts[:], state["pt_pair"][:], pt[:])
                            nc.tensor.matmul(den[:], ones_t[:], pts[:],
                                             start=(ci == 1), stop=(ci == cmax - 1))
                        state["pt_prev"] = pt

                    def tail_unit():
                        nc.tensor.matmul(pv[:], v_sb[:, NKC * b + cmax - 1, :],
                                         state["pt_prev"][:],
                                         start=(cmax == 1), stop=True)
                        # den rows are all identical (ones stationary)
                        den_rb = attev.tile([128, 512], F32, name="den_rb", tag="den_rb")
                        nc.vector.reciprocal_approx_fast(den_rb[:], den[:])
                        ao = attev.tile([128, 2, 2, 128], BF, name="ao", tag="ao")
                        nc.vector.tensor_mul(
                            ao[:].rearrange("p h u t -> p (h u t)"), pv[:], den_rb[:])
                        for u2 in range(2):
                            nc.sync.dma_start(
                                a2a_in[b][u2][qc].rearrange("(h p) t -> p h t", p=128),
                                ao[:, :, u2, :])

                    for ci in range(cmax):
                        yield lambda ci=ci: ci_unit(ci)
                    yield tail_unit

                ht8_tiles = [None] * NG

                def issue_ht8(g):
                    h8 = ht8_pool.tile([128, NHCP, 2, TN], F8, name="h8", tag="h8")
                    if g == 0:
                        # finer pieces so group 0's first matmuls start early
                        for q in range(NHCP):
                            nc.sync.dma_start(h8[:, q:q + 1],
                                              hT8_e[g, :, q:q + 1])
                    else:
                        nc.sync.dma_start(h8[:], hT8_e[g])
                    ht8_tiles[g] = h8

                def proj_units(g):
                    """Yield emission units for group g's projections."""
                    t0 = g * TN
                    hbt = [None]

                    def dma_unit():
                        # one batched DMA per tensor per group: the SP queue's
                        # ~585ns per-issue cost serializes fine-grained loads
                        # and starves the ao writes that gate the collectives.
                        # ht8 issues one group AHEAD (fp8 stream is the
                        # critical projection input).
                        if g == 0:
                            issue_ht8(0)
                        if g + 1 < NG:
                            issue_ht8(g + 1)
                        hb = htb_pool.tile([128, NHC, TN], BF, name="hb", tag="hb")
                        if g == 0:
                            for q in range(4):
                                nc.sync.dma_start(hb[:, 4 * q:4 * q + 4],
                                                  hTb_e[g, :, 4 * q:4 * q + 4])
                        else:
                            nc.sync.dma_start(hb[:], hTb_e[g])
                        hbt[0] = hb
                        if g in (2, 4):
                            half = (g - 2) // 2
                            nc.scalar.dma_start(
                                wo_res[half][:],
                                woT_e[:, half * 1024:(half + 1) * 1024]
                                .rearrange("(h p) n -> p h n", p=128))

                    ctab = g % (T // TN) * TN  # rope table column offset

                    def s_unit(s):
                        ps = psA.tile([128, TN], F32, name="psA", tag="psA")
                        if s < 3:
                            for c in range(NHCP):
                                nc.tensor.matmul(ps[:], w8_t[:, s, c],
                                                 ht8_tiles[g][:, c],
                                                 start=(c == 0), stop=(c == NHCP - 1),
                                                 perf_mode=DR)
                            # RoPE: out = ps*C + rot(ps)*S  (S carries the sign)
                            if s < 2:
                                dst = q_sb[:, s, t0:t0 + TN]
                            else:
                                dst = k_sb[:, t0:t0 + TN]
                            csl = ropeC_t[:, ctab:ctab + TN]
                            ssl = ropeS_t[:, ctab:ctab + TN]
                            # stage psum->bf16 on ACT: frees the psA slot after
                            # one read and lets every DVE op run in 2x 16-bit
                            # mode (~410ns vs ~690ns per [128,512])
                            rs = rtmp.tile([128, TN], BF, name="rs", tag="rs")
                            nc.scalar.copy(rs[:], ps[:])
                            t1 = rtmp.tile([128, TN], BF, name="t1", tag="t1")
                            t2 = rtmp.tile([128, TN], BF, name="t2", tag="t2")
                            # ropeS is stored partition-swapped ([sin, -sin])
                            # so each half-mul's SBUF inputs share a base
                            # partition (walrus NCC_IBIR297 constraint)
                            nc.vector.tensor_mul(t1[:], rs[:], csl)
                            nc.vector.tensor_mul(t2[0:64, :], rs[64:128, :], ssl[64:128, :])
                            nc.vector.tensor_mul(t2[64:128, :], rs[0:64, :], ssl[0:64, :])
                            nc.vector.tensor_add(dst, t1[:], t2[:])
                        else:
                            for hc in range(NHC):
                                nc.tensor.matmul(ps[:], wv_t[:, hc], hbt[0][:, hc],
                                                 start=(hc == 0), stop=(hc == NHC - 1))
                            # V^T -> transpose to V natural via PE (staging
                            # copy on ACT: DVE is the lagging engine and its
                            # backlog gates k_sb/strip tails)
                            vt = vtmp.tile([128, TN], BF, name="vt", tag="vt")
                            nc.scalar.copy(vt[:], ps[:])
                            for j in range(TN // 128):
                                trp = psTr.tile([128, 128], BF, name="trp", tag="trp")
                                nc.tensor.transpose(trp[:], vt[:, j * 128:(j + 1) * 128], ident_t[:])
                                nc.vector.tensor_copy(v_sb[:, g * (TN // 128) + j, :], trp[:])

                    yield dma_unit
                    for s in range(4):
                        yield lambda s=s: s_unit(s)

                def oproj_units(p):
                    """Yield emission units for o_proj of batch p (16 chunks)."""
                    def chunk_unit(tch, ch):
                        ps = psF_pool[0].tile([128, 512], F32, name="fin", tag="fin")
                        half, n2 = ch // 2, ch % 2
                        for h in range(NH):
                            nc.tensor.matmul(
                                ps[:],
                                att_g[p][:, h, tch * 128:(tch + 1) * 128],
                                wo_res[half][:, h, n2 * 512:(n2 + 1) * 512],
                                start=(h == 0), stop=(h == NH - 1))
                        fo = fo_pool_r[0].tile([128, 512], F32, name="fo", tag="fo")
                        if ch % 2 == 0:
                            nc.vector.tensor_copy(fo[:], ps[:])
                        else:
                            nc.scalar.copy(fo[:], ps[:])
                        # alternate output writes across both hwdge queues so
                        # the final drain halves
                        eng = nc.sync if ch % 2 == 0 else nc.scalar
                        eng.dma_start(
                            out_e[p * 256 + tch * 128: p * 256 + (tch + 1) * 128,
                                  (half * 1024 + n2 * 512):(half * 1024 + (n2 + 1) * 512)],
                            fo[:])

                    for tch in range(2):
                        for ch in range(4):
                            yield lambda tch=tch, ch=ch: chunk_unit(tch, ch)

                def zip_emit(primary, secondary):
                    """Interleave two unit streams ~evenly; primary-led."""
                    pu = list(primary)
                    su = list(secondary)
                    emitted = 0
                    for i, u in enumerate(pu):
                        u()
                        want = (i + 1) * len(su) // len(pu)
                        while emitted < want:
                            su[emitted]()
                            emitted += 1
                    while emitted < len(su):
                        su[emitted]()
                        emitted += 1

                def strips_for(g):
                    """Strips unlocked by group g's projections."""
                    if g < 0:
                        return []
                    if mode == "causal":
                        b = g // 4
                        return [(b, 2 * (g % 4)), (b, 2 * (g % 4) + 1)]
                    # non-causal strips read every key chunk of the batch
                    if g == 3:
                        return [(0, qc) for qc in range(NQC)]
                    if g == 7:
                        return [(1, qc) for qc in range(NQC)]
                    return []

                def strip_stream(g):
                    for b, qc in strips_for(g):
                        yield from strip_units(b, qc)

                # Rolling zipper: each group's strips enqueue after its k/v
                # are projected and drain interleaved into the NEXT group's
                # projection stream, keeping the PE queue free of exp stalls
                # while completing strips (and thus the AllToAlls) early.
                pending = []
                for g in range(NG):
                    rate = (len(pending) + 4) // 5
                    for u in proj_units(g):
                        u()
                        for _ in range(min(rate, len(pending))):
                            pending.pop(0)()
                    if g == 4:
                        # drain leftover batch-0 strips, then reshard batch 0
                        while pending:
                            pending.pop(0)()
                        for u2 in range(2):
                            nc.gpsimd.collective_compute(
                                "AllToAll", mybir.AluOpType.bypass,
                                replica_groups=[list(range(NC))],
                                ins=[a2a_in[0][u2][:].opt()],
                                outs=[a2a_out[0][u2][:].opt()])
                    pending.extend(strip_stream(g))
                    if g == NG - 1:
                        # gather batch-0 attention early (SP queue; a2a#0 is
                        # complete well before the queue reaches these)
                        att_g[0] = attgp.tile([128, NH, 256], BF, name="attg0",
                                              tag="attg")
                        for j in range(NC):
                            for u2 in range(2):
                                nc.sync.dma_start(
                                    att_g[0][:, 2 * j:2 * j + 2,
                                             128 * u2:128 * u2 + 128],
                                    a2a_out[0][u2][j].rearrange(
                                        "(h p) t -> p h t", p=128))

                es_proj.close()  # free projection pools (PSUM room for psF)

                # Tail: finish the last strips FIRST (their ao writes gate the
                # second AllToAll, shipped as two token-half collectives so
                # o_proj p1's first half starts while the second flies), with
                # o_proj p0 filling the PE meanwhile.
                with tc.tile_pool(name="psF", bufs=3, space="PSUM") as psF, \
                     tc.tile_pool(name="fo", bufs=3) as fo_pool:
                    psF_pool[0] = psF
                    fo_pool_r[0] = fo_pool
                    for u in pending:
                        u()
                    for u2 in range(2):
                        nc.gpsimd.collective_compute(
                            "AllToAll", mybir.AluOpType.bypass,
                            replica_groups=[list(range(NC))],
                            ins=[a2a_in[1][u2][:].opt()],
                            outs=[a2a_out[1][u2][:].opt()])
                    for u in oproj_units(0):
                        u()
                    att_g[1] = attgp.tile([128, NH, 256], BF, name="attg1", tag="attg")
                    for u2 in range(2):
                        for j in range(NC):
                            nc.sync.dma_start(
                                att_g[1][:, 2 * j:2 * j + 2,
                                         128 * u2:128 * u2 + 128],
                                a2a_out[1][u2][j].rearrange("(h p) t -> p h t", p=128))
                    for u in oproj_units(1):
                        u()

            es.close()
            es_wo.close()

    nc.compile()
    return nc


def _host_prep(hidden_states, freqs_cos, freqs_sin, mask, w_qkv, w_o, kv_write_indices):
    idx = np.asarray(kv_write_indices).astype(np.int64)
    if not np.array_equal(idx, np.arange(T, dtype=np.int64)):
        raise NotImplementedError("kernel specialized for kv_write_indices == arange(T)")

    hs = np.asarray(hidden_states, dtype=np.float32).reshape(TT, HID)
    hsT = hs.T  # [HID, TT]
    # bf16 copy (V path): [HID, TT] -> [NG, 128, NHC, TN] (one DMA per group,
    # 16KB contiguous per partition row)
    hTb = np.ascontiguousarray(
        hsT.reshape(NHC, 128, NG, TN).transpose(2, 1, 0, 3)).astype(NPBF)
    # fp8 copy (Q/K path, x64): [HID, TT] -> [NG, 128, NHCP, 2, TN]
    # element (g, p, c, kt, t) = hsT[128*(2c+kt)+p, g*TN+t] * 64
    h8full = (hsT * SC_H).reshape(NHCP, 2, 128, NG, TN).transpose(3, 2, 0, 1, 4)
    hT8 = np.ascontiguousarray(h8full).astype(NPF8)

    m2 = np.asarray(mask, dtype=np.float32).reshape(T, T)
    tril = np.tril(np.ones((T, T), dtype=bool))
    if not m2.any():
        mode = "none"
    elif (m2[tril] == 0).all() and (m2[~tril] <= -1e30).all():
        mode = "causal"
    else:
        mode = "generic"

    wq = np.asarray(w_qkv, dtype=np.float32)
    woT = np.ascontiguousarray(np.asarray(w_o, dtype=np.float32).T).astype(NPBF)

    def tile_w8(wrows):
        # [128 out, HID] -> [128 hid-in-pair? ...] stationary DoubleRow tiles:
        # element (p, c, kt, m) = wrows[m, 128*(2c+kt)+p]
        return wrows.T.reshape(NHCP, 2, 128, 128).transpose(2, 0, 1, 3)

    w8s = []
    wvs = []
    for c in range(NC):
        q1 = wq[(2 * c) * HD:(2 * c + 1) * HD] * (SCALE * SC_QW)
        q2 = wq[(2 * c + 1) * HD:(2 * c + 2) * HD] * (SCALE * SC_QW)
        k = wq[NH * HD + c * HD: NH * HD + (c + 1) * HD] * SC_KW
        v = wq[(NH + NKV) * HD + c * HD: (NH + NKV) * HD + (c + 1) * HD]
        # [3, NHCP, 2, 128] blocks -> [128 hid, 3, NHCP, 2, 128 out]
        w8s.append(np.ascontiguousarray(
            np.stack([tile_w8(q1), tile_w8(q2), tile_w8(k)], axis=1)
        ).astype(NPF8))
        # V: [128 out, HID] -> [NHC, 128 hid, 128 out] -> [128, NHC, 128]
        wvs.append(np.ascontiguousarray(
            v.T.reshape(NHC, 128, 128).transpose(1, 0, 2)).astype(NPBF))

    cosT = np.asarray(freqs_cos, dtype=np.float32).T  # [64, T]
    sinT = np.asarray(freqs_sin, dtype=np.float32).T
    ropeC = np.ascontiguousarray(np.concatenate([cosT, cosT], axis=0)).astype(NPBF)
    # partition-swapped: rows [0:64] hold +sin (used against x1 -> upper out),
    # rows [64:128] hold -sin (used against x2 -> lower out)
    ropeS = np.ascontiguousarray(np.concatenate([sinT, -sinT], axis=0)).astype(NPBF)

    consts = {
        "ropeC": ropeC,
        "ropeS": ropeS,
        "ones": np.ones((128, 128), NPBF),
        "ident": np.eye(128, dtype=np.float32).astype(NPBF),
    }
    if mode == "causal":
        kr = np.arange(256)[:, None]
        qr = np.arange(256)[None, :]
        pat = np.where(kr <= qr, np.float32(0.0), np.float32(NEG)).astype(np.float32)
        pat = pat.reshape(2, 128, 1, 256).repeat(2, axis=2)  # dup over heads
        consts["pat"] = np.ascontiguousarray(pat)
    elif mode == "generic":
        consts["maskT"] = np.ascontiguousarray(m2.T)

    in_maps = []
    for c in range(NC):
        m = {"hT8": hT8, "hTb": hTb, "w8": w8s[c], "wv": wvs[c], "woT": woT}
        m.update(consts)
        in_maps.append(m)
    return mode, in_maps


def kernel(hidden_states, freqs_cos, freqs_sin, k_cache, v_cache, mask, w_qkv,
           w_o, kv_write_indices):
    # k_cache/v_cache are fully overwritten (kv_write_indices == arange covers
    # every slot), so their incoming contents are irrelevant.
    global last_results
    mode, in_maps = _host_prep(hidden_states, freqs_cos, freqs_sin, mask,
                               w_qkv, w_o, kv_write_indices)
    if mode not in _CACHE:
        _CACHE[mode] = _build(mode)
    nc = _CACHE[mode]

    trace = bool(os.environ.get("BASS_KERNEL_TRACE"))
    res = run_bass_kernel_spmd(nc, in_maps, core_ids=list(range(NC)), trace=trace)
    last_results = res

    final = np.empty((B, T, HID), dtype=np.float32)
    for c in range(NC):
        o = res.results[c]["out"]
        final[0, 256 * c:256 * (c + 1)] = o[0:256]
        final[1, 256 * c:256 * (c + 1)] = o[256:512]
    return final
